# revision 22
# baseline (speedup 1.0000x reference)
"""Trainium2 Bass kernel for nn_ExplainableLSTM (2-layer LSTM, B=128 S=256 E=512 H=1024).

Strategy (8 NeuronCores, SPMD):
  - Front-end (embedding gather + input projection): data-parallel over batch
    (16 batches/core), producing x_in^T in [E, token] layout, then ONE
    AllGather replicates the full-batch projected inputs to every core.
  - Recurrence: tensor-parallel over the 4H gate dimension. Each core owns a
    128-row slice of H for each gate (i,f,o,g order), computes gates for the
    FULL batch of 128 (batch is the stationary operand's M dim -> full PE
    utilization), updates its c/h slice, and AllGathers h slices each step.
  - Output h1 slices are written per-step to DRAM; host assembles [B, S, H].

All compute is fp32 (matmul streams fp32 at the same rate as bf16 on trn2).
"""

import numpy as np

B, S_FULL, F, V, E, H = 128, 256, 4, 1024, 512, 1024
NCORES = 8
BL = B // NCORES          # 16 batches per core (front-end sharding)
GS = 4 * H // NCORES      # 512 gate rows per core ([i,f,o,g] x 128)
HS = H // NCORES          # 128 hidden rows per core
KT_E = E // 128           # 4 contraction tiles over E
KT_H = H // 128           # 8 contraction tiles over H
FP32 = None               # set in build_nc (mybir import kept local)


def build_nc(S=S_FULL):
    """Build the SPMD Bass program (same program on all 8 cores; per-core
    behavior differs only through input data: weight slices + index shards)."""
    import concourse.bass as bass
    import concourse.mybir as mybir
    import concourse.tile as tile
    from concourse import bacc
    from concourse.masks import make_identity

    fp32 = mybir.dt.float32
    AF = mybir.ActivationFunctionType

    assert S % 8 == 0
    ntok = S * BL             # tokens per core for the front-end
    ntt = ntok // 128         # 128-token tiles

    nc = bacc.Bacc(num_devices=NCORES)

    # ---------------- kernel I/O ----------------
    # idxf: token vocab ids as fp32 rows per feature; iota: per-partition
    # vocab values for the one-hot compare, iota[p, vt] = vt*128 + p.
    idxf_d = nc.dram_tensor("idxf", [F, ntok], fp32, kind="ExternalInput")
    iota_d = nc.dram_tensor("iota", [128, V // 128], fp32, kind="ExternalInput")
    emb_ds = [nc.dram_tensor(f"emb{f}", [V, E], fp32, kind="ExternalInput")
              for f in range(F)]
    pw_d = nc.dram_tensor("projw_t", [KT_E, 128, E], fp32, kind="ExternalInput")
    pb_d = nc.dram_tensor("projb_t", [128, KT_E], fp32, kind="ExternalInput")
    wih0_d = nc.dram_tensor("w_ih0t", [KT_E, 128, GS], fp32, kind="ExternalInput")
    whh0_d = nc.dram_tensor("w_hh0t", [KT_H, 128, GS], fp32, kind="ExternalInput")
    wih1_d = nc.dram_tensor("w_ih1t", [KT_H, 128, GS], fp32, kind="ExternalInput")
    whh1_d = nc.dram_tensor("w_hh1t", [KT_H, 128, GS], fp32, kind="ExternalInput")
    b0_d = nc.dram_tensor("b0row", [1, GS], fp32, kind="ExternalInput")
    b1_d = nc.dram_tensor("b1row", [1, GS], fp32, kind="ExternalInput")
    out_d = nc.dram_tensor("out", [S, 128, HS], fp32, kind="ExternalOutput")

    # internal DRAM for the x_in allgather
    xinT_d = nc.dram_tensor("xinT_local", [KT_E, 128, S, BL], fp32)
    X_d = nc.dram_tensor("Xgather", [NCORES, KT_E, 128, S, BL], fp32,
                         addr_space="Shared")

    RG = [list(range(NCORES))]

    with tile.TileContext(nc) as tc:
        # ---------------- persistent constants ----------------
        with (
            tc.tile_pool(name="const", bufs=1) as cpool,
            tc.tile_pool(name="wpool", bufs=1) as wpool,
            tc.tile_pool(name="state", bufs=1) as statep,
        ):
            identity = cpool.tile([128, 128], fp32)
            make_identity(nc, identity)
            ones1 = cpool.tile([1, 128], fp32)
            nc.vector.memset(ones1, 1.0)
            projb = cpool.tile([128, KT_E], fp32)
            nc.sync.dma_start(projb, pb_d[:, :])
            b0row = cpool.tile([1, GS], fp32)
            nc.sync.dma_start(b0row, b0_d[:, :])
            b1row = cpool.tile([1, GS], fp32)
            nc.sync.dma_start(b1row, b1_d[:, :])
            iota_sb = cpool.tile([128, V // 128], fp32)
            nc.sync.dma_start(iota_sb, iota_d[:, :])

            pw = wpool.tile([128, KT_E, E], fp32)
            nc.sync.dma_start(pw, pw_d.rearrange("k p e -> p k e"))
            wih0 = wpool.tile([128, KT_E, GS], fp32)
            nc.sync.dma_start(wih0, wih0_d.rearrange("k p g -> p k g"))
            whh0 = wpool.tile([128, KT_H, GS], fp32)
            nc.sync.dma_start(whh0, whh0_d.rearrange("k p g -> p k g"))
            wih1 = wpool.tile([128, KT_H, GS], fp32)
            nc.sync.dma_start(wih1, wih1_d.rearrange("k p g -> p k g"))
            whh1 = wpool.tile([128, KT_H, GS], fp32)
            nc.sync.dma_start(whh1, whh1_d.rearrange("k p g -> p k g"))

            # persistent cell state [batch-part, h-slice-free]
            c0 = statep.tile([128, HS], fp32)
            nc.vector.memset(c0, 0.0)
            c1 = statep.tile([128, HS], fp32)
            nc.vector.memset(c1, 0.0)

            # ---- front-end: one-hot matmul gather + sum + proj ----
            # embsumT[e', tok] = sum_f emb[f].T @ onehot_f.T, built per
            # 512-token chunk; onehot_f.T[v, t] = (idxf[f, t] == v).
            NVT = V // 128  # 8 vocab tiles
            TC = min(512, ntok)  # tokens per chunk
            with (
                tc.tile_pool(name="fe_emb", bufs=1) as fee,
                tc.tile_pool(name="fe_sb", bufs=2) as fes,
                tc.tile_pool(name="fe_ps", bufs=1, space="PSUM") as fep,
            ):
                emb_sb = []
                for f in range(F):
                    et = fee.tile([128, NVT, E], fp32, name=f"emb_sb{f}",
                                  tag=f"emb_sb{f}")
                    nc.sync.dma_start(et, emb_ds[f].rearrange(
                        "(vt p) e -> p vt e", p=128))
                    emb_sb.append(et)
                for tc_i in range(ntok // TC):
                    t0 = tc_i * TC
                    # embsumT PSUM tiles [e'-tile][128, TC]
                    esum = [fep.tile([128, TC], fp32, name=f"esum{m}",
                                     tag=f"esum{m}") for m in range(KT_E)]
                    first = [True] * KT_E
                    for f in range(F):
                        idxrep = fes.tile([128, TC], fp32, tag="idxrep")
                        nc.sync.dma_start(
                            idxrep,
                            idxf_d[f:f + 1, t0:t0 + TC].to_broadcast([128, TC]))
                        ohT = fes.tile([128, NVT, TC], fp32, tag="ohT", bufs=2)
                        for vt in range(NVT):
                            nc.vector.tensor_scalar(
                                out=ohT[:, vt, :], in0=idxrep,
                                scalar1=iota_sb[:, vt:vt + 1], scalar2=None,
                                op0=mybir.AluOpType.is_equal)
                        for m in range(KT_E):
                            for vt in range(NVT):
                                nc.tensor.matmul(
                                    esum[m],
                                    emb_sb[f][:, vt, m * 128:(m + 1) * 128],
                                    ohT[:, vt, :],
                                    start=first[m],
                                    stop=(f == F - 1 and vt == NVT - 1),
                                )
                                first[m] = False
                    esumT = fes.tile([128, KT_E, TC], fp32, tag="esumT")
                    for m in range(KT_E):
                        nc.vector.tensor_copy(esumT[:, m, :], esum[m])
                    # proj: xT[e, tok] = proj_w @ embsumT (+ proj_b)
                    for mt in range(KT_E):
                        pj = fep.tile([128, TC], fp32, tag="fepj")
                        for kt in range(KT_E):
                            nc.tensor.matmul(
                                pj, pw[:, kt, mt * 128:(mt + 1) * 128],
                                esumT[:, kt, :],
                                start=(kt == 0), stop=(kt == KT_E - 1),
                            )
                        xT = fes.tile([128, TC], fp32, tag="xT")
                        nc.scalar.activation(xT, pj, AF.Identity,
                                             bias=projb[:, mt:mt + 1])
                        nc.sync.dma_start(
                            xinT_d[mt, :, t0 // 16:(t0 + TC) // 16, :],
                            xT.rearrange("p (s b) -> p s b", b=BL),
                        )

            # ---------------- allgather x_in ----------------
            nc.gpsimd.collective_compute(
                "AllGather", mybir.AluOpType.bypass, replica_groups=RG,
                ins=[xinT_d.ap().opt()], outs=[X_d.ap().opt()],
            )

            # ---------------- recurrence ----------------
            with (
                tc.tile_pool(name="xst", bufs=4) as xpool,
                tc.tile_pool(name="hT", bufs=2) as hpool,
                tc.tile_pool(name="elt", bufs=2) as epool,
                tc.tile_pool(name="ps", bufs=2, space="PSUM") as ppool,
                tc.tile_pool(name="dr", bufs=2, space="DRAM") as dpool,
            ):
                h0T = None
                h1T = None
                for s in range(S):
                    # stationary x tiles for step s: [e-part, (rank, b)]
                    xstat = xpool.tile([128, KT_E, 128], fp32, tag="xstat")
                    for et in range(KT_E):
                        nc.sync.dma_start(
                            xstat[:, et, :].rearrange("p (r b) -> p r b", r=NCORES),
                            X_d[:, et, :, s, :].rearrange("r p b -> p r b"),
                        )

                    # ---- layer 0 gates ----
                    g0 = ppool.tile([128, GS], fp32, tag="g0")
                    nc.tensor.matmul(g0, ones1, b0row, start=True, stop=False)
                    n_mm = KT_E + (KT_H if s > 0 else 0)
                    i_mm = 0
                    for et in range(KT_E):
                        i_mm += 1
                        nc.tensor.matmul(g0, xstat[:, et, :], wih0[:, et, :],
                                         start=False, stop=(i_mm == n_mm))
                    if s > 0:
                        for kt in range(KT_H):
                            i_mm += 1
                            nc.tensor.matmul(g0, h0T[:, kt, :], whh0[:, kt, :],
                                             start=False, stop=(i_mm == n_mm))

                    # ---- layer 0 elementwise ----
                    sig0 = epool.tile([128, 384], fp32, tag="sig0")
                    nc.scalar.activation(sig0, g0[:, 0:384], AF.Sigmoid)
                    tg0 = epool.tile([128, HS], fp32, tag="tg0")
                    nc.scalar.activation(tg0, g0[:, 384:512], AF.Tanh)
                    t1 = epool.tile([128, HS], fp32, tag="t1")
                    nc.vector.tensor_mul(t1, sig0[:, 128:256], c0)
                    t2 = epool.tile([128, HS], fp32, tag="t2")
                    nc.vector.tensor_mul(t2, sig0[:, 0:128], tg0)
                    nc.vector.tensor_add(c0, t1, t2)
                    tc0 = epool.tile([128, HS], fp32, tag="tc0")
                    nc.scalar.activation(tc0, c0, AF.Tanh)
                    h0b = epool.tile([128, HS], fp32, tag="h0b")
                    nc.vector.tensor_mul(h0b, sig0[:, 256:384], tc0)

                    # ---- h0 slice -> transpose -> allgather ----
                    tr0 = ppool.tile([128, 128], fp32, tag="tr")
                    nc.tensor.transpose(tr0, h0b, identity)
                    h0s = epool.tile([128, 128], fp32, tag="h0s")
                    nc.vector.tensor_copy(h0s, tr0)
                    agi0 = dpool.tile([128, 128], fp32, tag="agi0")
                    nc.sync.dma_start(agi0, h0s)
                    ago0 = dpool.tile([NCORES, 128, 128], fp32, tag="ago0",
                                      addr_space="Shared")
                    nc.gpsimd.collective_compute(
                        "AllGather", mybir.AluOpType.bypass, replica_groups=RG,
                        ins=[agi0.opt()], outs=[ago0.opt()],
                    )
                    h0T = hpool.tile([128, KT_H, 128], fp32, tag="h0T")
                    nc.sync.dma_start(h0T, ago0.rearrange("r p b -> p r b"))

                    # ---- layer 1 gates ----
                    g1 = ppool.tile([128, GS], fp32, tag="g1")
                    nc.tensor.matmul(g1, ones1, b1row, start=True, stop=False)
                    n_mm = KT_H + (KT_H if s > 0 else 0)
                    i_mm = 0
                    for kt in range(KT_H):
                        i_mm += 1
                        nc.tensor.matmul(g1, h0T[:, kt, :], wih1[:, kt, :],
                                         start=False, stop=(i_mm == n_mm))
                    if s > 0:
                        for kt in range(KT_H):
                            i_mm += 1
                            nc.tensor.matmul(g1, h1T[:, kt, :], whh1[:, kt, :],
                                             start=False, stop=(i_mm == n_mm))

                    # ---- layer 1 elementwise ----
                    sig1 = epool.tile([128, 384], fp32, tag="sig1")
                    nc.scalar.activation(sig1, g1[:, 0:384], AF.Sigmoid)
                    tg1 = epool.tile([128, HS], fp32, tag="tg1")
                    nc.scalar.activation(tg1, g1[:, 384:512], AF.Tanh)
                    u1 = epool.tile([128, HS], fp32, tag="u1")
                    nc.vector.tensor_mul(u1, sig1[:, 128:256], c1)
                    u2 = epool.tile([128, HS], fp32, tag="u2")
                    nc.vector.tensor_mul(u2, sig1[:, 0:128], tg1)
                    nc.vector.tensor_add(c1, u1, u2)
                    tc1 = epool.tile([128, HS], fp32, tag="tc1")
                    nc.scalar.activation(tc1, c1, AF.Tanh)
                    h1b = epool.tile([128, HS], fp32, tag="h1b")
                    nc.vector.tensor_mul(h1b, sig1[:, 256:384], tc1)

                    # output slice for this step
                    nc.sync.dma_start(out_d[s, :, :], h1b)

                    # ---- h1 slice -> transpose -> allgather ----
                    if s < S - 1:
                        tr1 = ppool.tile([128, 128], fp32, tag="tr")
                        nc.tensor.transpose(tr1, h1b, identity)
                        h1s = epool.tile([128, 128], fp32, tag="h1s")
                        nc.vector.tensor_copy(h1s, tr1)
                        agi1 = dpool.tile([128, 128], fp32, tag="agi1")
                        nc.sync.dma_start(agi1, h1s)
                        ago1 = dpool.tile([NCORES, 128, 128], fp32, tag="ago1",
                                          addr_space="Shared")
                        nc.gpsimd.collective_compute(
                            "AllGather", mybir.AluOpType.bypass,
                            replica_groups=RG,
                            ins=[agi1.opt()], outs=[ago1.opt()],
                        )
                        h1T = hpool.tile([128, KT_H, 128], fp32, tag="h1T")
                        nc.sync.dma_start(h1T, ago1.rearrange("r p b -> p r b"))

    nc.compile()
    return nc


def prep_inputs(inputs, S=S_FULL):
    """Host-side prep: per-core input maps (weight slices, wrapped indices)."""
    x = np.asarray(inputs["x"])[:, :S, :]
    emb = np.asarray(inputs["emb"], dtype=np.float32)
    proj_w = np.asarray(inputs["proj_w"], dtype=np.float32)
    proj_b = np.asarray(inputs["proj_b"], dtype=np.float32)
    W_ih0 = np.asarray(inputs["W_ih0"], dtype=np.float32)
    W_hh0 = np.asarray(inputs["W_hh0"], dtype=np.float32)
    W_ih1 = np.asarray(inputs["W_ih1"], dtype=np.float32)
    W_hh1 = np.asarray(inputs["W_hh1"], dtype=np.float32)
    b0 = np.asarray(inputs["b_ih0"], dtype=np.float32) + np.asarray(
        inputs["b_hh0"], dtype=np.float32)
    b1 = np.asarray(inputs["b_ih1"], dtype=np.float32) + np.asarray(
        inputs["b_hh1"], dtype=np.float32)

    pw_t = np.ascontiguousarray(proj_w.T).reshape(KT_E, 128, E)
    pb_t = np.ascontiguousarray(proj_b.reshape(KT_E, 128).T)

    in_maps = []
    for r in range(NCORES):
        # gate-row selection for this core, [i, f, o, g] blocks of 128
        rows = np.arange(r * HS, (r + 1) * HS)
        sel = np.concatenate([rows, H + rows, 3 * H + rows, 2 * H + rows])

        # token t = s*BL + b; idxf[f, t] = x[r*BL + b, s, f]
        xs = x[r * BL:(r + 1) * BL]  # [16, S, F]
        idxf = np.ascontiguousarray(
            xs.transpose(2, 1, 0).reshape(F, S * BL).astype(np.float32))
        iota = np.ascontiguousarray(
            np.arange(V, dtype=np.float32).reshape(V // 128, 128).T)

        in_maps.append({
            "idxf": idxf,
            "iota": iota,
            **{f"emb{f}": np.ascontiguousarray(emb[f]) for f in range(F)},
            "projw_t": pw_t,
            "projb_t": pb_t,
            "w_ih0t": np.ascontiguousarray(W_ih0[sel].T).reshape(KT_E, 128, GS),
            "w_hh0t": np.ascontiguousarray(W_hh0[sel].T).reshape(KT_H, 128, GS),
            "w_ih1t": np.ascontiguousarray(W_ih1[sel].T).reshape(KT_H, 128, GS),
            "w_hh1t": np.ascontiguousarray(W_hh1[sel].T).reshape(KT_H, 128, GS),
            "b0row": b0[sel].reshape(1, GS),
            "b1row": b1[sel].reshape(1, GS),
        })
    return in_maps


def assemble_output(results, S=S_FULL):
    out = np.empty((B, S, H), dtype=np.float32)
    for r in range(NCORES):
        o = np.asarray(results[r]["out"]).reshape(S, 128, HS)
        out[:, :, r * HS:(r + 1) * HS] = o.transpose(1, 0, 2)
    return out


def kernel(**inputs) -> np.ndarray:
    from concourse.bass_utils import run_bass_kernel_spmd

    nc = build_nc(S_FULL)
    in_maps = prep_inputs(inputs, S_FULL)
    res = run_bass_kernel_spmd(nc, in_maps, core_ids=list(range(NCORES)))
    return assemble_output(res.results, S_FULL)


if __name__ == "__main__":
    # smoke test with random data
    rng = np.random.default_rng(0)
    ins = {
        "x": rng.integers(0, V, (B, S_FULL, F)),
        "emb": rng.standard_normal((F, V, E), dtype=np.float32) * 0.05,
        "proj_w": rng.standard_normal((E, E), dtype=np.float32) * 0.05,
        "proj_b": np.zeros(E, np.float32),
        "W_ih0": rng.standard_normal((4 * H, E), dtype=np.float32) * 0.05,
        "W_hh0": rng.standard_normal((4 * H, H), dtype=np.float32) * 0.05,
        "b_ih0": np.zeros(4 * H, np.float32),
        "b_hh0": np.zeros(4 * H, np.float32),
        "W_ih1": rng.standard_normal((4 * H, H), dtype=np.float32) * 0.05,
        "W_hh1": rng.standard_normal((4 * H, H), dtype=np.float32) * 0.05,
        "b_ih1": np.zeros(4 * H, np.float32),
        "b_hh1": np.zeros(4 * H, np.float32),
    }
    out = kernel(**ins)
    print(out.shape, out.dtype, np.abs(out).mean())
